# revision 2
# baseline (speedup 1.0000x reference)
"""Trainium2 Bass kernel for nn_NeuralNetwork_27745488732940 (gnn_message_passing).

Topology (hardcoded from the problem spec): a layered DAG of 7 levels
(1024 -> 4096 x6 -> 512), each target neuron has K=512 random incoming
edges from the previous level; the whole network is LINEAR (bias+tanh only
at the output level).

Strategy:
  - Each level is a sparse matvec msg = A_l @ state_{l-1} with fixed fan-in.
    Trainium has no line-rate gather engine, so we evaluate each level as a
    DENSE matvec on the TensorEngine: the host scatters (weights, edge_from)
    into the dense per-level matrix A_l^T (a pure re-layout of the given
    arrays; all FLOPs stay on device).
  - Shard the target rows of every level across the 8 NeuronCores (each core
    computes 1/8 of each level's outputs = its "partial segment_sum" chunk),
    then AllGather the 2KB state chunks per level (replicated state vector).
  - PE matvec form: stationary = 128-element state k-tile (one column),
    moving = A^T k-tile [128 x rows]; PSUM accumulates over k-tiles.
"""

import numpy as np

# ---- static genome topology ----
I_, H, O_, KFAN, NHID = 1024, 4096, 512, 512, 6
LEVEL_SIZES = [I_] + [H] * NHID + [O_]
LEVEL_STARTS = np.cumsum([0] + LEVEL_SIZES).tolist()
N_NEUR = LEVEL_STARTS[-1]
EDGE_COUNTS = [LEVEL_SIZES[l + 1] * KFAN for l in range(len(LEVEL_SIZES) - 1)]
NCORES = 8
NLEVELS = len(LEVEL_SIZES) - 1  # 7

DTYPE = "float32"  # "float32" | "bfloat16"

_module_cache = {}


def _np_dtype():
    if DTYPE == "bfloat16":
        import ml_dtypes

        return ml_dtypes.bfloat16
    return np.float32


def _build_module():
    import concourse.mybir as mybir
    import concourse.tile as tile
    from concourse import bacc

    mdt = mybir.dt.bfloat16 if DTYPE == "bfloat16" else mybir.dt.float32
    f32 = mybir.dt.float32

    nc = bacc.Bacc(
        "TRN2",
        target_bir_lowering=False,
        debug=False,
        enable_asserts=False,
        num_devices=NCORES,
    )

    # ---- I/O declarations (per-core shapes) ----
    a_dram = []
    for li in range(NLEVELS):
        S = LEVEL_SIZES[li]
        rc = LEVEL_SIZES[li + 1] // NCORES
        nk = S // 128
        a_dram.append(
            nc.dram_tensor(f"a{li}", [nk, 128, rc], mdt, kind="ExternalInput")
        )
    x_dram = nc.dram_tensor("x", [I_], f32, kind="ExternalInput")
    b_dram = nc.dram_tensor("b", [O_ // NCORES], f32, kind="ExternalInput")
    out_dram = nc.dram_tensor("out", [O_ // NCORES], f32, kind="ExternalOutput")

    rg = [list(range(NCORES))]

    with tile.TileContext(nc) as tc:
        with (
            tc.tile_pool(name="a_pool", bufs=2) as a_pool,
            tc.tile_pool(name="state", bufs=2) as state_pool,
            tc.tile_pool(name="chunk", bufs=2) as chunk_pool,
            tc.tile_pool(name="ps", bufs=2, space="PSUM") as psum_pool,
            tc.tile_pool(name="dram", bufs=2, space="DRAM") as dram_pool,
        ):
            # load x -> state0 [128, 8]  (state[s] at partition s//nk, col s%nk)
            nk = I_ // 128
            st = state_pool.tile([128, nk], mdt, tag="st")
            if DTYPE == "bfloat16":
                nc.gpsimd.dma_start(
                    st[:, :], x_dram.ap().rearrange("(p j) -> p j", j=nk)
                )
            else:
                nc.sync.dma_start(
                    st[:, :], x_dram.ap().rearrange("(p j) -> p j", j=nk)
                )

            for li in range(NLEVELS):
                S = LEVEL_SIZES[li]
                rc = LEVEL_SIZES[li + 1] // NCORES
                nk = S // 128
                a_t = a_pool.tile([128, nk * rc], mdt, tag="a")
                nc.sync.dma_start(
                    a_t[:, :].rearrange("p (j r) -> p j r", j=nk),
                    a_dram[li].ap().rearrange("j p r -> p j r"),
                )
                ps = psum_pool.tile([1, rc], f32, tag="ps")
                for j in range(nk):
                    nc.tensor.matmul(
                        ps[:, :],
                        st[:, j : j + 1],
                        a_t[:, j * rc : (j + 1) * rc],
                        start=(j == 0),
                        stop=(j == nk - 1),
                    )
                if li < NLEVELS - 1:
                    # chunk -> DRAM -> AllGather -> next state tile
                    sb = chunk_pool.tile([1, rc], mdt, tag="sb")
                    nc.vector.tensor_copy(sb[:, :], ps[:, :])
                    cc_in = dram_pool.tile([1, rc], mdt, tag="ccin")
                    cc_out = dram_pool.tile([1, rc * NCORES], mdt, tag="ccout")
                    nc.gpsimd.dma_start(cc_in[:, :], sb[:, :])
                    nc.gpsimd.collective_compute(
                        "AllGather",
                        mybir.AluOpType.bypass,
                        replica_groups=rg,
                        ins=[cc_in.opt()],
                        outs=[cc_out.opt()],
                    )
                    S2 = LEVEL_SIZES[li + 1]
                    nk2 = S2 // 128
                    st = state_pool.tile([128, nk2], mdt, tag="st")
                    nc.sync.dma_start(
                        st[:, :],
                        cc_out[0, :].rearrange("(p j) -> p j", j=nk2),
                    )
                else:
                    # bias + tanh -> out
                    bias_sb = chunk_pool.tile([1, rc], f32, tag="bias")
                    nc.sync.dma_start(
                        bias_sb[:, :], b_dram.ap().rearrange("(o r) -> o r", o=1)
                    )
                    out_sb = chunk_pool.tile([1, rc], f32, tag="outsb")
                    nc.vector.tensor_add(out_sb[:, :], ps[:, :], bias_sb[:, :])
                    nc.scalar.activation(
                        out_sb[:, :],
                        out_sb[:, :],
                        mybir.ActivationFunctionType.Tanh,
                    )
                    nc.sync.dma_start(
                        out_dram.ap().rearrange("(o r) -> o r", o=1), out_sb[:, :]
                    )

    nc.compile()
    return nc


def _prep_inputs(x, weights, biases, edge_from, edge_to):
    """Host-side: densify each level's edges into A^T and shard by target."""
    npdt = _np_dtype()
    per_core = [dict() for _ in range(NCORES)]
    off = 0
    for li in range(NLEVELS):
        S = LEVEL_SIZES[li]
        T = LEVEL_SIZES[li + 1]
        rc = T // NCORES
        nk = S // 128
        e_cnt = EDGE_COUNTS[li]
        ef = np.asarray(edge_from[off : off + e_cnt], dtype=np.int64) - LEVEL_STARTS[li]
        et = (
            np.asarray(edge_to[off : off + e_cnt], dtype=np.int64)
            - LEVEL_STARTS[li + 1]
        )
        w = np.asarray(weights[off : off + e_cnt], dtype=np.float64)
        off += e_cnt
        flat = ef * T + et
        a_full = np.bincount(flat, weights=w, minlength=S * T).reshape(S, T)
        # s = nk*p + j  ->  A3[j, p, r] = A^T[nk*p + j, r]
        for c in range(NCORES):
            chunk = a_full[:, c * rc : (c + 1) * rc]
            a3 = (
                chunk.reshape(128, nk, rc)
                .transpose(1, 0, 2)
                .astype(npdt)
                .copy(order="C")
            )
            per_core[c][f"a{li}"] = a3
    xb = np.asarray(x, dtype=np.float32)
    bb = np.asarray(biases, dtype=np.float32)[N_NEUR - O_ :]
    for c in range(NCORES):
        per_core[c]["x"] = xb
        per_core[c]["b"] = np.ascontiguousarray(bb[c * (O_ // NCORES) : (c + 1) * (O_ // NCORES)])
    return per_core


def kernel(x, weights, biases, edge_from, edge_to, _profile=None):
    from concourse.bass_utils import run_bass_kernel_spmd

    if "nc" not in _module_cache:
        _module_cache["nc"] = _build_module()
    nc = _module_cache["nc"]

    in_maps = _prep_inputs(x, weights, biases, edge_from, edge_to)
    kwargs = dict(_profile) if _profile else {}
    res = run_bass_kernel_spmd(nc, in_maps, core_ids=list(range(NCORES)), **kwargs)
    out = np.concatenate([res.results[c]["out"] for c in range(NCORES)])
    if _profile is not None:
        _module_cache["last_results"] = res
    return out.astype(np.float32)


# revision 5
# speedup vs baseline: 1.4933x; 1.4933x over previous
"""Trainium2 Bass kernel for nn_NeuralNetwork_27745488732940 (gnn_message_passing).

Topology (hardcoded from the problem spec): a layered DAG of 7 levels
(1024 -> 4096 x6 -> 512), each target neuron has K=512 random incoming
edges from the previous level; the whole network is LINEAR (bias+tanh only
at the output level).

Strategy:
  - Each level is a sparse matvec msg = A_l @ state_{l-1} with fixed fan-in.
    Trainium has no line-rate gather engine, so we evaluate each level as a
    DENSE matvec on the TensorEngine: the host scatters (weights, edge_from)
    into the dense per-level matrix A_l^T (a pure re-layout of the given
    arrays; all FLOPs stay on device).
  - Shard the target rows of every level across the 8 NeuronCores (each core
    computes 1/8 of each level's outputs = its "partial segment_sum" chunk),
    then AllGather the 2KB state chunks per level (replicated state vector).
  - PE matvec form: stationary = 128-element state k-tile (one column),
    moving = A^T k-tile [128 x rows]; PSUM accumulates over k-tiles.
"""

import numpy as np

# ---- static genome topology ----
I_, H, O_, KFAN, NHID = 1024, 4096, 512, 512, 6
LEVEL_SIZES = [I_] + [H] * NHID + [O_]
LEVEL_STARTS = np.cumsum([0] + LEVEL_SIZES).tolist()
N_NEUR = LEVEL_STARTS[-1]
EDGE_COUNTS = [LEVEL_SIZES[l + 1] * KFAN for l in range(len(LEVEL_SIZES) - 1)]
NCORES = 8
NLEVELS = len(LEVEL_SIZES) - 1  # 7

DTYPE = "bfloat16"  # "float32" | "bfloat16"

_module_cache = {}


def _np_dtype():
    if DTYPE == "bfloat16":
        import ml_dtypes

        return ml_dtypes.bfloat16
    return np.float32


def _build_module():
    import concourse.mybir as mybir
    import concourse.tile as tile
    from concourse import bacc

    mdt = mybir.dt.bfloat16 if DTYPE == "bfloat16" else mybir.dt.float32
    f32 = mybir.dt.float32

    nc = bacc.Bacc(
        "TRN2",
        target_bir_lowering=False,
        debug=False,
        enable_asserts=False,
        num_devices=NCORES,
    )

    # ---- I/O declarations (per-core shapes) ----
    a_dram = []
    for li in range(NLEVELS):
        S = LEVEL_SIZES[li]
        rc = LEVEL_SIZES[li + 1] // NCORES
        nk = S // 128
        a_dram.append(
            nc.dram_tensor(f"a{li}", [nk, 128, rc], mdt, kind="ExternalInput")
        )
    x_dram = nc.dram_tensor("x", [I_], f32, kind="ExternalInput")
    b_dram = nc.dram_tensor("b", [O_ // NCORES], f32, kind="ExternalInput")
    out_dram = nc.dram_tensor("out", [O_ // NCORES], f32, kind="ExternalOutput")

    rg = [list(range(NCORES))]

    with tile.TileContext(nc) as tc:
        with (
            tc.tile_pool(name="a_pool", bufs=2) as a_pool,
            tc.tile_pool(name="state", bufs=2) as state_pool,
            tc.tile_pool(name="chunk", bufs=2) as chunk_pool,
            tc.tile_pool(name="ps", bufs=2, space="PSUM") as psum_pool,
            tc.tile_pool(name="dram", bufs=2, space="DRAM") as dram_pool,
        ):
            # Dummy AllGather at t=0: absorbs the one-time collectives
            # entry barrier (~50us of core-start skew) under the prologue
            # DMAs instead of stalling the first real exchange.
            dummy_in = dram_pool.tile([1, 4], f32, tag="dummy_in")
            dummy_out = dram_pool.tile([1, 4 * NCORES], f32, tag="dummy_out")
            nc.gpsimd.collective_compute(
                "AllGather",
                mybir.AluOpType.bypass,
                replica_groups=rg,
                ins=[dummy_in.opt()],
                outs=[dummy_out.opt()],
            )

            # load x -> state0 [128, 8]  (state[s] at partition s//nk, col s%nk)
            nk = I_ // 128
            st = state_pool.tile([128, nk], mdt, tag="st")
            if DTYPE == "bfloat16":
                nc.gpsimd.dma_start(
                    st[:, :], x_dram.ap().rearrange("(p j) -> p j", j=nk)
                )
            else:
                nc.sync.dma_start(
                    st[:, :], x_dram.ap().rearrange("(p j) -> p j", j=nk)
                )

            for li in range(NLEVELS):
                S = LEVEL_SIZES[li]
                rc = LEVEL_SIZES[li + 1] // NCORES
                nk = S // 128
                a_t = a_pool.tile([128, nk * rc], mdt, tag="a")
                nc.sync.dma_start(
                    a_t[:, :].rearrange("p (j r) -> p j r", j=nk),
                    a_dram[li].ap().rearrange("j p r -> p j r"),
                )
                ps = psum_pool.tile([1, rc], f32, tag="ps")
                for j in range(nk):
                    nc.tensor.matmul(
                        ps[:, :],
                        st[:, j : j + 1],
                        a_t[:, j * rc : (j + 1) * rc],
                        start=(j == 0),
                        stop=(j == nk - 1),
                    )
                if li < NLEVELS - 1:
                    # chunk -> DRAM -> AllGather -> next state tile
                    sb = chunk_pool.tile([1, rc], mdt, tag="sb")
                    nc.vector.tensor_copy(sb[:, :], ps[:, :])
                    cc_in = dram_pool.tile([1, rc], mdt, tag="ccin")
                    cc_out = dram_pool.tile([1, rc * NCORES], mdt, tag="ccout")
                    nc.sync.dma_start(cc_in[:, :], sb[:, :])
                    nc.gpsimd.collective_compute(
                        "AllGather",
                        mybir.AluOpType.bypass,
                        replica_groups=rg,
                        ins=[cc_in.opt()],
                        outs=[cc_out.opt()],
                    )
                    S2 = LEVEL_SIZES[li + 1]
                    nk2 = S2 // 128
                    st = state_pool.tile([128, nk2], mdt, tag="st")
                    nc.sync.dma_start(
                        st[:, :],
                        cc_out[0, :].rearrange("(p j) -> p j", j=nk2),
                    )
                else:
                    # bias + tanh -> out
                    bias_sb = chunk_pool.tile([1, rc], f32, tag="bias")
                    nc.sync.dma_start(
                        bias_sb[:, :], b_dram.ap().rearrange("(o r) -> o r", o=1)
                    )
                    out_sb = chunk_pool.tile([1, rc], f32, tag="outsb")
                    nc.vector.tensor_add(out_sb[:, :], ps[:, :], bias_sb[:, :])
                    nc.scalar.activation(
                        out_sb[:, :],
                        out_sb[:, :],
                        mybir.ActivationFunctionType.Tanh,
                    )
                    nc.sync.dma_start(
                        out_dram.ap().rearrange("(o r) -> o r", o=1), out_sb[:, :]
                    )

    nc.compile()
    return nc


def _prep_inputs(x, weights, biases, edge_from, edge_to):
    """Host-side: densify each level's edges into A^T and shard by target."""
    npdt = _np_dtype()
    per_core = [dict() for _ in range(NCORES)]
    off = 0
    for li in range(NLEVELS):
        S = LEVEL_SIZES[li]
        T = LEVEL_SIZES[li + 1]
        rc = T // NCORES
        nk = S // 128
        e_cnt = EDGE_COUNTS[li]
        ef = np.asarray(edge_from[off : off + e_cnt], dtype=np.int64) - LEVEL_STARTS[li]
        et = (
            np.asarray(edge_to[off : off + e_cnt], dtype=np.int64)
            - LEVEL_STARTS[li + 1]
        )
        w = np.asarray(weights[off : off + e_cnt], dtype=np.float64)
        off += e_cnt
        flat = ef * T + et
        a_full = np.bincount(flat, weights=w, minlength=S * T).reshape(S, T)
        # s = nk*p + j  ->  A3[j, p, r] = A^T[nk*p + j, r]
        for c in range(NCORES):
            chunk = a_full[:, c * rc : (c + 1) * rc]
            a3 = (
                chunk.reshape(128, nk, rc)
                .transpose(1, 0, 2)
                .astype(npdt)
                .copy(order="C")
            )
            per_core[c][f"a{li}"] = a3
    xb = np.asarray(x, dtype=np.float32)
    bb = np.asarray(biases, dtype=np.float32)[N_NEUR - O_ :]
    for c in range(NCORES):
        per_core[c]["x"] = xb
        per_core[c]["b"] = np.ascontiguousarray(bb[c * (O_ // NCORES) : (c + 1) * (O_ // NCORES)])
    return per_core


def kernel(x, weights, biases, edge_from, edge_to, _profile=None):
    from concourse.bass_utils import run_bass_kernel_spmd

    if "nc" not in _module_cache:
        _module_cache["nc"] = _build_module()
    nc = _module_cache["nc"]

    in_maps = _prep_inputs(x, weights, biases, edge_from, edge_to)
    kwargs = dict(_profile) if _profile else {}
    res = run_bass_kernel_spmd(nc, in_maps, core_ids=list(range(NCORES)), **kwargs)
    out = np.concatenate([res.results[c]["out"] for c in range(NCORES)])
    if _profile is not None:
        _module_cache["last_results"] = res
    return out.astype(np.float32)


# revision 6
# speedup vs baseline: 1.7559x; 1.1759x over previous
"""Trainium2 Bass kernel for nn_NeuralNetwork_27745488732940 (gnn_message_passing).

Topology (hardcoded from the problem spec): a layered DAG of 7 levels
(1024 -> 4096 x6 -> 512), each target neuron has K=512 random incoming
edges from the previous level; the whole network is LINEAR (bias+tanh only
at the output level).

Strategy:
  - Each level is a sparse matvec msg = A_l @ state_{l-1} with fixed fan-in.
    Trainium has no line-rate gather engine, so we evaluate each level as a
    DENSE matvec on the TensorEngine: the host scatters (weights, edge_from)
    into the dense per-level matrix A_l^T (a pure re-layout of the given
    arrays; all FLOPs stay on device).
  - Shard the target rows of every level across the 8 NeuronCores (each core
    computes 1/8 of each level's outputs = its "partial segment_sum" chunk),
    then AllGather the 2KB state chunks per level (replicated state vector).
  - PE matvec form: stationary = 128-element state k-tile (one column),
    moving = A^T k-tile [128 x rows]; PSUM accumulates over k-tiles.
  - Small control DMAs (chunk export, state assembly) ride the ACT HWDGE
    ring so they never queue behind the multi-MB A-matrix stream on the SP
    ring.
"""

import numpy as np

# ---- static genome topology ----
I_, H, O_, KFAN, NHID = 1024, 4096, 512, 512, 6
LEVEL_SIZES = [I_] + [H] * NHID + [O_]
LEVEL_STARTS = np.cumsum([0] + LEVEL_SIZES).tolist()
N_NEUR = LEVEL_STARTS[-1]
EDGE_COUNTS = [LEVEL_SIZES[l + 1] * KFAN for l in range(len(LEVEL_SIZES) - 1)]
NCORES = 8
NLEVELS = len(LEVEL_SIZES) - 1  # 7

DTYPE = "bfloat16"  # "float32" | "bfloat16"
KT_CHUNK = 8  # k-tiles per A-DMA chunk (finer DMA/compute pipelining)

_module_cache = {}


def _np_dtype():
    if DTYPE == "bfloat16":
        import ml_dtypes

        return ml_dtypes.bfloat16
    return np.float32


def _build_module():
    import concourse.mybir as mybir
    import concourse.tile as tile
    from concourse import bacc

    mdt = mybir.dt.bfloat16 if DTYPE == "bfloat16" else mybir.dt.float32
    f32 = mybir.dt.float32

    nc = bacc.Bacc(
        "TRN2",
        target_bir_lowering=False,
        debug=False,
        enable_asserts=False,
        num_devices=NCORES,
    )

    # ---- I/O declarations (per-core shapes) ----
    # a{li}: [128, nk, rc] — partition-major so each partition's read is one
    # contiguous nk*rc run.
    a_dram = []
    for li in range(NLEVELS):
        S = LEVEL_SIZES[li]
        rc = LEVEL_SIZES[li + 1] // NCORES
        nk = S // 128
        a_dram.append(
            nc.dram_tensor(f"a{li}", [128, nk, rc], mdt, kind="ExternalInput")
        )
    x_dram = nc.dram_tensor("x", [I_], f32, kind="ExternalInput")
    b_dram = nc.dram_tensor("b", [O_ // NCORES], f32, kind="ExternalInput")
    out_dram = nc.dram_tensor("out", [O_ // NCORES], f32, kind="ExternalOutput")

    rg = [list(range(NCORES))]

    with tile.TileContext(nc) as tc:
        with (
            tc.tile_pool(name="a_pool", bufs=6) as a_pool,
            tc.tile_pool(name="state", bufs=2) as state_pool,
            tc.tile_pool(name="chunk", bufs=2) as chunk_pool,
            tc.tile_pool(name="ps", bufs=2, space="PSUM") as psum_pool,
            tc.tile_pool(name="dram", bufs=2, space="DRAM") as dram_pool,
        ):
            # Dummy AllGather at t=0: absorbs the one-time collectives
            # entry barrier (~40-50us of core-start skew / ncfw init) under
            # the prologue DMAs instead of stalling the first real exchange.
            dummy_in = dram_pool.tile([1, 4], f32, tag="dummy_in")
            dummy_out = dram_pool.tile([1, 4 * NCORES], f32, tag="dummy_out")
            nc.gpsimd.collective_compute(
                "AllGather",
                mybir.AluOpType.bypass,
                replica_groups=rg,
                ins=[dummy_in.opt()],
                outs=[dummy_out.opt()],
            )

            # load x -> state0 [128, 8]  (state[s] at partition s//nk, col s%nk)
            nk = I_ // 128
            st = state_pool.tile([128, nk], mdt, tag="st")
            if DTYPE == "bfloat16":
                nc.gpsimd.dma_start(
                    st[:, :], x_dram.ap().rearrange("(p j) -> p j", j=nk)
                )
            else:
                nc.scalar.dma_start(
                    st[:, :], x_dram.ap().rearrange("(p j) -> p j", j=nk)
                )

            for li in range(NLEVELS):
                S = LEVEL_SIZES[li]
                rc = LEVEL_SIZES[li + 1] // NCORES
                nk = S // 128
                ps = psum_pool.tile([1, rc], f32, tag="ps")
                # A-matrix streamed in KT_CHUNK k-tile chunks on the SP ring;
                # matmuls chase each chunk as it lands.
                for j0 in range(0, nk, KT_CHUNK):
                    jn = min(KT_CHUNK, nk - j0)
                    a_t = a_pool.tile([128, jn * rc], mdt, tag="a")
                    nc.sync.dma_start(
                        a_t[:, :],
                        a_dram[li][:, j0 : j0 + jn, :].rearrange("p j r -> p (j r)"),
                    )
                    for dj in range(jn):
                        j = j0 + dj
                        nc.tensor.matmul(
                            ps[:, :],
                            st[:, j : j + 1],
                            a_t[:, dj * rc : (dj + 1) * rc],
                            start=(j == 0),
                            stop=(j == nk - 1),
                        )
                if li < NLEVELS - 1:
                    # chunk -> DRAM -> AllGather -> next state tile
                    sb = chunk_pool.tile([1, rc], mdt, tag="sb")
                    nc.vector.tensor_copy(sb[:, :], ps[:, :])
                    cc_in = dram_pool.tile([1, rc], mdt, tag="ccin")
                    cc_out = dram_pool.tile([1, rc * NCORES], mdt, tag="ccout")
                    nc.scalar.dma_start(cc_in[:, :], sb[:, :])
                    nc.gpsimd.collective_compute(
                        "AllGather",
                        mybir.AluOpType.bypass,
                        replica_groups=rg,
                        ins=[cc_in.opt()],
                        outs=[cc_out.opt()],
                    )
                    S2 = LEVEL_SIZES[li + 1]
                    nk2 = S2 // 128
                    st = state_pool.tile([128, nk2], mdt, tag="st")
                    nc.scalar.dma_start(
                        st[:, :],
                        cc_out[0, :].rearrange("(p j) -> p j", j=nk2),
                    )
                else:
                    # bias + tanh -> out
                    bias_sb = chunk_pool.tile([1, rc], f32, tag="bias")
                    nc.scalar.dma_start(
                        bias_sb[:, :], b_dram.ap().rearrange("(o r) -> o r", o=1)
                    )
                    out_sb = chunk_pool.tile([1, rc], f32, tag="outsb")
                    nc.vector.tensor_add(out_sb[:, :], ps[:, :], bias_sb[:, :])
                    nc.scalar.activation(
                        out_sb[:, :],
                        out_sb[:, :],
                        mybir.ActivationFunctionType.Tanh,
                    )
                    nc.scalar.dma_start(
                        out_dram.ap().rearrange("(o r) -> o r", o=1), out_sb[:, :]
                    )

    nc.compile()
    return nc


def _prep_inputs(x, weights, biases, edge_from, edge_to):
    """Host-side: densify each level's edges into A^T and shard by target."""
    npdt = _np_dtype()
    per_core = [dict() for _ in range(NCORES)]
    off = 0
    for li in range(NLEVELS):
        S = LEVEL_SIZES[li]
        T = LEVEL_SIZES[li + 1]
        rc = T // NCORES
        nk = S // 128
        e_cnt = EDGE_COUNTS[li]
        ef = np.asarray(edge_from[off : off + e_cnt], dtype=np.int64) - LEVEL_STARTS[li]
        et = (
            np.asarray(edge_to[off : off + e_cnt], dtype=np.int64)
            - LEVEL_STARTS[li + 1]
        )
        w = np.asarray(weights[off : off + e_cnt], dtype=np.float64)
        off += e_cnt
        flat = ef * T + et
        a_full = np.bincount(flat, weights=w, minlength=S * T).reshape(S, T)
        # state[s] lives at (partition p = s // nk, col j = s % nk);
        # a{li}[p, j, r] = A^T[nk*p + j, r]
        for c in range(NCORES):
            chunk = a_full[:, c * rc : (c + 1) * rc]
            a3 = chunk.reshape(128, nk, rc).astype(npdt).copy(order="C")
            per_core[c][f"a{li}"] = a3
    xb = np.asarray(x, dtype=np.float32)
    bb = np.asarray(biases, dtype=np.float32)[N_NEUR - O_ :]
    for c in range(NCORES):
        per_core[c]["x"] = xb
        per_core[c]["b"] = np.ascontiguousarray(
            bb[c * (O_ // NCORES) : (c + 1) * (O_ // NCORES)]
        )
    return per_core


def kernel(x, weights, biases, edge_from, edge_to, _profile=None):
    from concourse.bass_utils import run_bass_kernel_spmd

    if "nc" not in _module_cache:
        _module_cache["nc"] = _build_module()
    nc = _module_cache["nc"]

    in_maps = _prep_inputs(x, weights, biases, edge_from, edge_to)
    kwargs = dict(_profile) if _profile else {}
    res = run_bass_kernel_spmd(nc, in_maps, core_ids=list(range(NCORES)), **kwargs)
    out = np.concatenate([res.results[c]["out"] for c in range(NCORES)])
    if _profile is not None:
        _module_cache["last_results"] = res
    return out.astype(np.float32)
